# revision 8
# baseline (speedup 1.0000x reference)
"""Trainium2 Bass kernel for nn_AtlasApan (TGN scatter-memory update).

Strategy (8 NeuronCores, SPMD):
  - Batch dim (24576) sharded contiguously: 3072 nodes/core = 24 tiles of 128.
  - `mail` [50000,10,288] replicated in each core's HBM; the per-node mailbox
    gather (11.5KB/node) runs on-device via indirect DMA — that gather is the
    dominant memory traffic.
  - Host precomputes tiny per-node tensors: time features (no cos on ACT),
    slot-validity masks (the reference's modulo reorder reduces to a mask
    because softmax attention is permutation invariant), gathered prev-memory
    in both node-major and feature-major layouts.
  - Math folds: b_k cancels in softmax; LayerNorm affine folds into the MLP
    weights; K/V share one fused matmul (rhs = [w_k.T | w_v.T]); projection
    biases ride a ones-row in the contraction.
Per 128-node tile on device:
  gather mail -> PE-transpose to feature-major msgT -> K|V matmuls into PSUM
  -> scores = per-node dot(Q,K) via DVE mul + segmented reduce -> masked
  softmax (exp on ACT, no max-sub needed; empty rows renormalize to 0)
  -> out = attn-weighted V -> +prev_mem -> LN (ln/exp rsqrt trick) -> MLP+relu.
"""

import numpy as np

P = 128
NT = 24            # tiles per core
S = 10             # mail slots
DM = 288           # mail message dim
DTIME = 64         # time-encoding dim
D = 128            # embed dim
NN = 50000         # nodes in memory
NCORES = 8
B_CORE = NT * P    # 3072
N_TOTAL = NCORES * B_CORE  # 24576

_CACHE = {}


def _build():
    import concourse.mybir as mybir
    from concourse.bacc import Bacc
    from concourse.tile import TileContext
    from concourse.tile_rust import add_dep_helper
    from concourse.masks import make_identity
    from concourse.bass import IndirectOffsetOnAxis, AP

    dt = mybir.dt
    f32 = dt.float32
    i32 = dt.int32
    Alu = mybir.AluOpType
    Act = mybir.ActivationFunctionType
    Ax = mybir.AxisListType

    nc = Bacc()

    mail = nc.dram_tensor("mail", [NN, S * DM], f32, kind="ExternalInput")
    idxs = nc.dram_tensor("idxs", [P, NT], i32, kind="ExternalInput")
    # [tile, 64 time rows + 1 ones row, 10*128] feature-major time features
    tfd = nc.dram_tensor("tfd", [NT, DTIME + 1, S * P], f32, kind="ExternalInput")
    memT = nc.dram_tensor("memT", [P, NT * P], f32, kind="ExternalInput")
    prevm = nc.dram_tensor("prevm", [P, NT * P], f32, kind="ExternalInput")
    maskf = nc.dram_tensor("maskf", [P, NT * S], f32, kind="ExternalInput")
    wkv0 = nc.dram_tensor("wkv0", [P, 256], f32, kind="ExternalInput")
    wkv1 = nc.dram_tensor("wkv1", [P, 256], f32, kind="ExternalInput")
    wkv2 = nc.dram_tensor("wkv2", [97, 256], f32, kind="ExternalInput")
    wqT = nc.dram_tensor("wqT", [P, P], f32, kind="ExternalInput")
    bq = nc.dram_tensor("bq", [1, P], f32, kind="ExternalInput")
    mlpT = nc.dram_tensor("mlpT", [P, P], f32, kind="ExternalInput")
    mlpb = nc.dram_tensor("mlpb", [1, P], f32, kind="ExternalInput")
    out = nc.dram_tensor("out", [NT * P, P], f32, kind="ExternalOutput")

    def bcast(ap, n, pos):
        """Insert a broadcast (step 0, count n) free dim at position pos."""
        dims = [list(d) for d in ap.ap]
        dims.insert(pos, [0, n])
        return AP(ap.tensor, ap.offset, dims)

    with TileContext(nc) as tc:
        with tc.tile_pool(name="const", bufs=1) as cp, \
             tc.tile_pool(name="gat", bufs=3) as gp, \
             tc.tile_pool(name="work", bufs=2) as wp, \
             tc.tile_pool(name="small", bufs=3) as sp, \
             tc.tile_pool(name="stg", bufs=3, space="PSUM") as pstg, \
             tc.tile_pool(name="kvp", bufs=1, space="PSUM") as pkv:

            ident = cp.tile([P, P], f32)
            make_identity(nc, ident[:])
            ones1 = cp.tile([1, P], f32)
            nc.gpsimd.memset(ones1[:], 1.0)
            # consts used as activation biases
            zeroc = cp.tile([P, 1], f32)
            nc.gpsimd.memset(zeroc[:], 0.0)
            nc.const_aps.aps[(f32, 0.0)] = zeroc[:]
            epsc = cp.tile([P, 1], f32)
            nc.gpsimd.memset(epsc[:], 1e-5)
            nc.const_aps.aps[(f32, 1e-5)] = epsc[:]

            idx_s = cp.tile([P, NT], i32)
            nc.sync.dma_start(out=idx_s[:], in_=idxs[:])
            maskf_s = cp.tile([P, NT * S], f32)
            nc.sync.dma_start(out=maskf_s[:], in_=maskf[:])
            prevm_s = cp.tile([P, NT * P], f32)
            nc.sync.dma_start(out=prevm_s[:], in_=prevm[:])
            memT_s = cp.tile([P, NT * P], f32)
            nc.sync.dma_start(out=memT_s[:], in_=memT[:])
            wkv0_s = cp.tile([P, 256], f32)
            nc.sync.dma_start(out=wkv0_s[:], in_=wkv0[:])
            wkv1_s = cp.tile([P, 256], f32)
            nc.sync.dma_start(out=wkv1_s[:], in_=wkv1[:])
            wkv2_s = cp.tile([97, 256], f32)
            nc.sync.dma_start(out=wkv2_s[:], in_=wkv2[:])
            wqT_s = cp.tile([P, P], f32)
            nc.sync.dma_start(out=wqT_s[:], in_=wqT[:])
            bq_s = cp.tile([1, P], f32)
            nc.sync.dma_start(out=bq_s[:], in_=bq[:])
            mlpT_s = cp.tile([P, P], f32)
            nc.sync.dma_start(out=mlpT_s[:], in_=mlpT[:])
            mlpb_s = cp.tile([1, P], f32)
            nc.sync.dma_start(out=mlpb_s[:], in_=mlpb[:])

            for t in range(NT):
                # ---- gather mail rows for this tile's 128 nodes ----
                mailg = gp.tile([P, S * DM], f32, tag="mailg")
                nc.gpsimd.indirect_dma_start(
                    out=mailg[:],
                    out_offset=None,
                    in_=mail[:],
                    in_offset=IndirectOffsetOnAxis(ap=idx_s[:, t:t + 1], axis=0),
                )

                # ---- assemble feature-major msgT (3 contraction tiles) ----
                msgT0 = wp.tile([P, S * P], f32, tag="m0")
                msgT1 = wp.tile([P, S * P], f32, tag="m1")
                msgT2 = wp.tile([97, S * P], f32, tag="m2")
                # time features + ones row straight from DRAM
                nc.sync.dma_start(out=msgT2[32:97, :], in_=tfd[t])

                # PSUM group discipline: start=True lazily zeroes the whole
                # 2KB bank, so exactly one accumulation group per bank —
                # start on the bank's first write, stop on its last, with
                # explicit ordering deps between the group's matmuls.
                for c in range(3):
                    dstt = (msgT0, msgT1, msgT2)[c]
                    cw = 128 if c < 2 else 32
                    for g in range(3):  # slot groups 0-3, 4-7, 8-9
                        js = list(range(g * 4, min(g * 4 + 4, S)))
                        stg = pstg.tile([P, 512], f32, tag="stg")
                        prev = None
                        for k, j in enumerate(js):
                            mi = nc.tensor.matmul(
                                out=stg[0:cw, k * P:(k + 1) * P],
                                lhsT=mailg[:, j * DM + c * 128: j * DM + c * 128 + cw],
                                rhs=ident[:],
                                is_transpose=True,
                                start=(k == 0),
                                stop=(k == len(js) - 1),
                            )
                            if prev is not None:
                                add_dep_helper(mi.ins, prev.ins, sync=False,
                                               reason="psum bank group order")
                            prev = mi
                        gw = len(js) * P
                        nc.scalar.copy(
                            out=dstt[0:cw, g * 512: g * 512 + gw],
                            in_=stg[0:cw, 0:gw],
                        )

                # ---- K|V projections: kv[:, j*256:+128]=K_j, +128:+256=V_j ----
                kv = pkv.tile([P, S * 256], f32, tag="kv")
                mts = [(msgT0, wkv0_s, P), (msgT1, wkv1_s, P), (msgT2, wkv2_s, 97)]
                for b in range(S // 2):  # one psum bank per slot pair
                    prev = None
                    for j in (2 * b, 2 * b + 1):
                        for c, (mt, ws, rows) in enumerate(mts):
                            mi = nc.tensor.matmul(
                                out=kv[:, j * 256:(j + 1) * 256],
                                lhsT=mt[0:rows, j * P:(j + 1) * P],
                                rhs=ws[:],
                                start=(j == 2 * b and c == 0),
                                stop=(j == 2 * b + 1 and c == 2),
                            )
                            if prev is not None:
                                add_dep_helper(mi.ins, prev.ins, sync=False,
                                               reason="psum bank group order")
                            prev = mi

                # ---- Q = memT.T @ wqT + b_q ----
                qp = pstg.tile([P, 512], f32, tag="stg")
                q1 = nc.tensor.matmul(out=qp[:, 0:P], lhsT=memT_s[:, t * P:(t + 1) * P],
                                      rhs=wqT_s[:], start=True, stop=False)
                q2 = nc.tensor.matmul(out=qp[:, 0:P], lhsT=ones1[:], rhs=bq_s[:],
                                      start=False, stop=True)
                add_dep_helper(q2.ins, q1.ins, sync=False, reason="psum group order")
                qs = sp.tile([P, P], f32, tag="qs")
                nc.scalar.copy(out=qs[:], in_=qp[:, 0:P])

                # ---- scores l[n,j,h] = sum_d Q[n,h,d] K[n,j,h,d] ----
                kview = kv[:].rearrange("p (j kd) -> p j kd", j=S)[:, :, 0:P]
                prod = wp.tile([P, S * P], f32, tag="prod")
                prod3 = prod[:].rearrange("p (j d) -> p j d", j=S)
                nc.vector.tensor_tensor(out=prod3, in0=kview,
                                        in1=bcast(qs[:], S, 1), op=Alu.mult)
                lt = sp.tile([P, S * 2], f32, tag="lt")
                nc.vector.reduce_sum(
                    out=lt[:].rearrange("p (j h) -> p j h", j=S),
                    in_=prod[:].rearrange("p (j h d) -> p j h d", j=S, h=2),
                    axis=Ax.X)

                # ---- masked softmax over slots (no max-sub; |l|/8 is small) ----
                e = sp.tile([P, S * 2], f32, tag="e")
                nc.scalar.activation(out=e[:], in_=lt[:], func=Act.Exp, scale=0.125)
                em = sp.tile([P, S * 2], f32, tag="em")
                mslice = maskf_s[:, t * S:(t + 1) * S]
                nc.vector.tensor_tensor(
                    out=em[:].rearrange("p (j h) -> p j h", j=S),
                    in0=e[:].rearrange("p (j h) -> p j h", j=S),
                    in1=bcast(mslice, 2, 2), op=Alu.mult)
                ssum = sp.tile([P, 2], f32, tag="ssum")
                nc.vector.reduce_sum(
                    out=ssum[:],
                    in_=em[:].rearrange("p (j h) -> p h j", j=S), axis=Ax.X)
                sc = sp.tile([P, 2], f32, tag="sc")
                nc.vector.tensor_scalar_max(sc[:], ssum[:], 1e-6)
                rr = sp.tile([P, 2], f32, tag="rr")
                nc.vector.reciprocal(rr[:], sc[:])
                attn = sp.tile([P, S * 2], f32, tag="attn")
                nc.vector.tensor_tensor(
                    out=attn[:].rearrange("p (j h) -> p j h", j=S),
                    in0=em[:].rearrange("p (j h) -> p j h", j=S),
                    in1=bcast(rr[:], S, 1), op=Alu.mult)

                # ---- out_v[n,h,d] = sum_j attn[n,j,h] V[n,j,h,d] ----
                vview = kv[:].rearrange("p (j two hd) -> p j two hd", j=S, two=2)[:, :, 1, :] \
                    .rearrange("p j (h d) -> p j h d", h=2)
                prod2 = wp.tile([P, S * P], f32, tag="prod2")
                nc.vector.tensor_tensor(
                    out=prod2[:].rearrange("p (j h d) -> p j h d", j=S, h=2),
                    in0=vview,
                    in1=bcast(attn[:].rearrange("p (j h) -> p j h", j=S), DTIME, 3),
                    op=Alu.mult)
                outv = sp.tile([P, P], f32, tag="outv")
                nc.vector.reduce_sum(
                    out=outv[:],
                    in_=prod2[:].rearrange("p (j hd) -> p hd j", j=S), axis=Ax.X)

                # ---- residual + LayerNorm (affine folded into MLP) ----
                x = sp.tile([P, P], f32, tag="x")
                nc.vector.tensor_tensor(out=x[:], in0=outv[:],
                                        in1=prevm_s[:, t * P:(t + 1) * P], op=Alu.add)
                xsum = sp.tile([P, 1], f32, tag="xsum")
                nc.vector.reduce_sum(out=xsum[:], in_=x[:], axis=Ax.X)
                negmu = sp.tile([P, 1], f32, tag="negmu")
                nc.vector.tensor_scalar_mul(negmu[:], xsum[:], -1.0 / D)
                xc = sp.tile([P, P], f32, tag="xc")
                nc.vector.tensor_scalar_add(xc[:], x[:], negmu[:])
                sq = sp.tile([P, P], f32, tag="sq")
                nc.vector.tensor_tensor(out=sq[:], in0=xc[:], in1=xc[:], op=Alu.mult)
                vsum = sp.tile([P, 1], f32, tag="vsum")
                nc.vector.reduce_sum(out=vsum[:], in_=sq[:], axis=Ax.X)
                lnv = sp.tile([P, 1], f32, tag="lnv")
                nc.scalar.activation(out=lnv[:], in_=vsum[:], func=Act.Ln,
                                     bias=1e-5, scale=1.0 / D)
                rstd = sp.tile([P, 1], f32, tag="rstd")
                nc.scalar.activation(out=rstd[:], in_=lnv[:], func=Act.Exp,
                                     scale=-0.5)
                xn = sp.tile([P, P], f32, tag="xn")
                nc.vector.tensor_scalar_mul(xn[:], xc[:], rstd[:])

                # ---- MLP + relu ----
                xtp = pstg.tile([P, 512], f32, tag="stg")
                nc.tensor.transpose(out=xtp[:, 0:P], in_=xn[:], identity=ident[:])
                xts = sp.tile([P, P], f32, tag="xts")
                nc.scalar.copy(out=xts[:], in_=xtp[:, 0:P])
                yp = pstg.tile([P, 512], f32, tag="stg")
                y1 = nc.tensor.matmul(out=yp[:, 0:P], lhsT=xts[:], rhs=mlpT_s[:],
                                      start=True, stop=False)
                y2 = nc.tensor.matmul(out=yp[:, 0:P], lhsT=ones1[:], rhs=mlpb_s[:],
                                      start=False, stop=True)
                add_dep_helper(y2.ins, y1.ins, sync=False, reason="psum group order")
                upd = sp.tile([P, P], f32, tag="upd")
                nc.scalar.activation(out=upd[:], in_=yp[:, 0:P], func=Act.Relu)
                nc.sync.dma_start(out=out[t * P:(t + 1) * P, :], in_=upd[:])

    nc.compile()
    return nc


def _host_prep(inputs):
    """Build per-core in_maps from full inputs."""
    memory = np.asarray(inputs["memory"], dtype=np.float32)
    mail = np.asarray(inputs["mail"], dtype=np.float32)
    mail_ts = np.asarray(inputs["mail_ts"], dtype=np.float32)
    mail_ptr = np.asarray(inputs["mail_ptr"], dtype=np.int32)
    mail_count = np.asarray(inputs["mail_count"], dtype=np.int32)
    nodes = np.asarray(inputs["nodes"], dtype=np.int32)
    curr_ts = np.asarray(inputs["curr_ts"], dtype=np.float32)
    w_k = np.asarray(inputs["w_k"], dtype=np.float32)
    w_v = np.asarray(inputs["w_v"], dtype=np.float32)
    b_v = np.asarray(inputs["b_v"], dtype=np.float32)
    w_q = np.asarray(inputs["w_q"], dtype=np.float32)
    b_q = np.asarray(inputs["b_q"], dtype=np.float32)
    mlp_w = np.asarray(inputs["mlp_w"], dtype=np.float32)
    mlp_b = np.asarray(inputs["mlp_b"], dtype=np.float32)
    ln_g = np.asarray(inputs["ln_g"], dtype=np.float32)
    ln_b = np.asarray(inputs["ln_b"], dtype=np.float32)
    time_w = np.asarray(inputs["time_w"], dtype=np.float32)
    time_b = np.asarray(inputs["time_b"], dtype=np.float32)

    mail2d = np.ascontiguousarray(mail.reshape(NN, S * DM))

    # shared weights
    wkv = np.concatenate([w_k.T, w_v.T], axis=1).astype(np.float32)  # [352, 256]
    bias_row = np.concatenate([np.zeros(P, np.float32), b_v])[None, :]  # b_k cancels
    wkv2 = np.ascontiguousarray(np.concatenate([wkv[256:352], bias_row], axis=0))
    wqT = np.ascontiguousarray(w_q.T)
    mlpT = np.ascontiguousarray(ln_g[:, None] * mlp_w.T)
    mlpb = np.ascontiguousarray((ln_b @ mlp_w.T + mlp_b)[None, :])

    shared = {
        "mail": mail2d,
        "wkv0": np.ascontiguousarray(wkv[0:128]),
        "wkv1": np.ascontiguousarray(wkv[128:256]),
        "wkv2": wkv2,
        "wqT": wqT,
        "bq": np.ascontiguousarray(b_q[None, :]),
        "mlpT": mlpT,
        "mlpb": mlpb,
    }

    in_maps = []
    jj = np.arange(S, dtype=np.int32)
    for c in range(NCORES):
        sl = slice(c * B_CORE, (c + 1) * B_CORE)
        nd = nodes[sl]
        ct = curr_ts[sl]
        mts = mail_ts[nd]                        # [3072, 10]
        dtv = ct[:, None] - mts                  # [3072, 10]
        tf = np.cos(dtv[:, :, None] * time_w + time_b).astype(np.float32)
        # -> [NT, 64, 10*128] feature-major, cols j*128+n; append ones row
        tfT = tf.reshape(NT, P, S, DTIME).transpose(0, 3, 2, 1)
        tfd = np.concatenate(
            [tfT, np.ones((NT, 1, S, P), np.float32)], axis=1
        ).reshape(NT, DTIME + 1, S * P)

        ptr = mail_ptr[nd]
        cnt = mail_count[nd]
        mask = (((ptr[:, None] - 1 - jj) % S) < cnt[:, None]).astype(np.float32)
        maskf = np.ascontiguousarray(
            mask.reshape(NT, P, S).transpose(1, 0, 2).reshape(P, NT * S))

        pm = memory[nd]                          # [3072, 128]
        prevm = np.ascontiguousarray(
            pm.reshape(NT, P, D).transpose(1, 0, 2).reshape(P, NT * D))
        memT = np.ascontiguousarray(
            pm.reshape(NT, P, D).transpose(2, 0, 1).reshape(D, NT * P))
        idxs = np.ascontiguousarray(nd.reshape(NT, P).T)

        m = dict(shared)
        m.update({
            "idxs": idxs,
            "tfd": np.ascontiguousarray(tfd),
            "memT": memT,
            "prevm": prevm,
            "maskf": maskf,
        })
        in_maps.append(m)
    return in_maps


def kernel(**inputs):
    from concourse.bass_utils import run_bass_kernel_spmd

    nodes = np.asarray(inputs["nodes"], dtype=np.int32)
    curr_ts = np.asarray(inputs["curr_ts"], dtype=np.float32)
    memory = np.asarray(inputs["memory"], dtype=np.float32)
    memory_ts = np.asarray(inputs["memory_ts"], dtype=np.float32)
    assert nodes.shape[0] == N_TOTAL

    if "nc" not in _CACHE:
        _CACHE["nc"] = _build()
    nc = _CACHE["nc"]

    in_maps = _host_prep(inputs)
    import os
    trace = bool(int(os.environ.get("KERNEL_TRACE", "0")))
    res = run_bass_kernel_spmd(nc, in_maps, core_ids=list(range(NCORES)),
                               trace=trace)
    _CACHE["last_result"] = res

    upd = np.concatenate([r["out"] for r in res.results], axis=0)  # [24576, 128]

    bb = N_TOTAL // 3
    pos = nodes[:2 * bb]
    new_memory = memory.copy()
    new_memory[pos] = upd[:2 * bb]
    new_memory_ts = memory_ts.copy()
    new_memory_ts[pos] = curr_ts[:2 * bb]
    return upd, new_memory, new_memory_ts


# revision 16
# speedup vs baseline: 1.7857x; 1.7857x over previous
"""Trainium2 Bass kernel for nn_AtlasApan (TGN scatter-memory update).

Strategy (8 NeuronCores, SPMD):
  - Batch dim (24576) sharded contiguously: 3072 nodes/core = 24 tiles of 128.
  - `mail` (first 256 msg dims, bf16) replicated in each core's HBM; the
    per-node mailbox gather runs on-device via indirect DMA.
  - Host precomputes small per-node tensors: time features + last 32 mail
    dims, pre-transposed (feature-major), slot-validity masks (the
    reference's modulo reorder reduces to a mask because softmax attention
    is permutation invariant), prev-memory in both layouts.
  - Math folds: b_k cancels in softmax; LayerNorm affine folds into the MLP
    weights; rstd commutes through the MLP matmul (applied in a fused
    scalar_tensor_tensor after it); K/V share one fused matmul
    (rhs = [w_k.T | w_v.T]); rsqrt via bit-trick + Newton (avoids ACT
    table-set thrash between Exp and Ln).
Per 128-node tile on device (all matmul operands bf16, PSUM fp32):
  gather mail -> PE-transpose to feature-major msgT -> K|V matmuls into
  PSUM (one accumulation group per 2KB bank) -> copy K,V to SBUF bf16 ->
  per-node scores via DVE mul+segmented reduce -> masked softmax (exp on
  ACT, no max-sub needed; empty rows renormalize to 0) -> attn-weighted V
  -> +prev_mem -> LN stats via bn_stats -> MLP + relu.
"""

import numpy as np

P = 128
NT = 24            # tiles per core
S = 10             # mail slots
DM = 288           # mail message dim
DMD = 256          # device-gathered mail dims (rest rides with tfd)
DTIME = 64         # time-encoding dim
D = 128            # embed dim
NN = 50000         # nodes in memory
NCORES = 8
B_CORE = NT * P    # 3072
N_TOTAL = NCORES * B_CORE  # 24576

_CACHE = {}


def _build():
    import concourse.mybir as mybir
    from concourse.bacc import Bacc
    from concourse.tile import TileContext
    from concourse.tile_rust import add_dep_helper
    from concourse.masks import make_identity
    from concourse.bass import IndirectOffsetOnAxis, AP

    dt = mybir.dt
    f32 = dt.float32
    bf16 = dt.bfloat16
    i32 = dt.int32
    u32 = dt.uint32
    Alu = mybir.AluOpType
    Act = mybir.ActivationFunctionType
    Ax = mybir.AxisListType

    nc = Bacc()

    mail = nc.dram_tensor("mail", [NN, S * DMD], bf16, kind="ExternalInput")
    idxs = nc.dram_tensor("idxs", [P, NT], i32, kind="ExternalInput")
    # [tile, 32 mail-c2 rows + 64 time rows + 1 ones row, 10*128]
    tfd = nc.dram_tensor("tfd", [NT, 97, S * P], bf16, kind="ExternalInput")
    # packed bf16 consts: memT | wkv0 | wkv1 | wkv2(pad128) | wqT | mlpT | mask
    CB = NT * P + 256 + 256 + 256 + 128 + 128 + NT * S  # 4336
    constb = nc.dram_tensor("constb", [P, CB], bf16, kind="ExternalInput")
    # packed f32 consts: prevm | mlpbrep
    CF = NT * P + 128
    constf = nc.dram_tensor("constf", [P, CF], f32, kind="ExternalInput")
    bq = nc.dram_tensor("bq", [1, P], bf16, kind="ExternalInput")
    out = nc.dram_tensor("out", [NT * P, P], f32, kind="ExternalOutput")

    def bcast(ap, n, pos):
        """Insert a broadcast (step 0, count n) free dim at position pos."""
        dims = [list(d) for d in ap.ap]
        dims.insert(pos, [0, n])
        return AP(ap.tensor, ap.offset, dims)

    MAGIC = 0x5F3759DF

    with TileContext(nc) as tc:
        with tc.tile_pool(name="const", bufs=1) as cp, \
             tc.tile_pool(name="gat", bufs=3) as gp, \
             tc.tile_pool(name="work", bufs=2) as wp, \
             tc.tile_pool(name="small", bufs=3) as sp, \
             tc.tile_pool(name="stgp", bufs=2, space="PSUM") as pstg, \
             tc.tile_pool(name="smp", bufs=1, space="PSUM") as psmall, \
             tc.tile_pool(name="kvp", bufs=1, space="PSUM") as pkv:

            identb = cp.tile([P, P], bf16)
            make_identity(nc, identb[:])
            ones1b = cp.tile([1, P], bf16)
            nc.gpsimd.memset(ones1b[:], 1.0)
            zeroc = cp.tile([P, 1], f32)
            nc.gpsimd.memset(zeroc[:], 0.0)
            nc.const_aps.aps[(f32, 0.0)] = zeroc[:]
            magicc = cp.tile([P, 1], u32)
            nc.gpsimd.memset(magicc[:], MAGIC)

            cb = cp.tile([P, CB], bf16)
            nc.sync.dma_start(out=cb[:], in_=constb[:])
            cf = cp.tile([P, CF], f32)
            nc.sync.dma_start(out=cf[:], in_=constf[:])
            bq_s = cp.tile([1, P], bf16)
            nc.sync.dma_start(out=bq_s[:], in_=bq[:])
            idx_s = cp.tile([P, NT], i32)
            nc.sync.dma_start(out=idx_s[:], in_=idxs[:])

            o = NT * P
            memT_s = cb
            wkv0_o = o; o += 256
            wkv1_o = o; o += 256
            wkv2_o = o; o += 256
            wqT_o = o; o += 128
            mlpT_o = o; o += 128
            mask_o = o
            wkv_os = (wkv0_o, wkv1_o, wkv2_o)

            for t in range(NT):
                # ---- gather mail rows for this tile's 128 nodes ----
                mailg = gp.tile([P, S * DMD], bf16, tag="mailg")
                nc.gpsimd.indirect_dma_start(
                    out=mailg[:],
                    out_offset=None,
                    in_=mail[:],
                    in_offset=IndirectOffsetOnAxis(ap=idx_s[:, t:t + 1], axis=0),
                )

                # ---- assemble feature-major msgT (bf16) ----
                msgT0 = wp.tile([P, S * P], bf16, tag="m0")
                msgT1 = wp.tile([P, S * P], bf16, tag="m1")
                msgT2 = gp.tile([97, S * P], bf16, tag="m2")
                nc.sync.dma_start(out=msgT2[:], in_=tfd[t])

                # transpose groups fill a full PSUM bank, then one/two ACT
                # copies move them to SBUF. group entries: (chunk, slot list,
                # stg col base); copy entries: (dst, dst col, stg col, width)
                plans = [
                    ([(0, range(0, 8), 0)],
                     [(msgT0, 0, 0, 1024)]),
                    ([(0, range(8, 10), 0), (1, range(0, 6), 256)],
                     [(msgT0, 1024, 0, 256), (msgT1, 0, 256, 768)]),
                    ([(1, range(6, 10), 0)],
                     [(msgT1, 768, 0, 512)]),
                ]
                for grp, cps in plans:
                    stg = pstg.tile([P, 1024], bf16, tag="stg")
                    nmm = sum(len(sl) for _, sl, _ in grp)
                    k = 0
                    prev = None
                    for c, sl, base in grp:
                        for ji, j in enumerate(sl):
                            mi = nc.tensor.matmul(
                                out=stg[:, base + ji * P: base + (ji + 1) * P],
                                lhsT=mailg[:, j * DMD + c * 128: j * DMD + c * 128 + 128],
                                rhs=identb[:],
                                is_transpose=True,
                                start=(k == 0),
                                stop=(k == nmm - 1),
                            )
                            if prev is not None:
                                add_dep_helper(mi.ins, prev.ins, sync=False,
                                               reason="psum bank group order")
                            prev = mi
                            k += 1
                    for dst, dco, sco, wdt in cps:
                        nc.scalar.copy(out=dst[:, dco:dco + wdt],
                                       in_=stg[:, sco:sco + wdt])

                # ---- K|V projections: kv[:, j*256:+128]=K_j, +128:+256=V_j ----
                kv = pkv.tile([P, S * 256], f32, tag="kv")
                srcs = [(msgT0[:], P), (msgT1[:], P), (msgT2[:], 97)]
                for b in range(S // 2):  # one psum bank per slot pair
                    prev = None
                    for j in (2 * b, 2 * b + 1):
                        for c, (mt, rows) in enumerate(srcs):
                            mi = nc.tensor.matmul(
                                out=kv[:, j * 256:(j + 1) * 256],
                                lhsT=mt[0:rows, j * P:(j + 1) * P],
                                rhs=cb[0:rows, wkv_os[c]:wkv_os[c] + 256],
                                start=(j == 2 * b and c == 0),
                                stop=(j == 2 * b + 1 and c == 2),
                            )
                            if prev is not None:
                                add_dep_helper(mi.ins, prev.ins, sync=False,
                                               reason="psum bank group order")
                            prev = mi

                # copy K, V out to SBUF as bf16 (frees PSUM, enables 2x DVE)
                ks = wp.tile([P, S * P], bf16, tag="ks")
                nc.scalar.copy(
                    out=ks[:].rearrange("p (j d) -> p j d", j=S),
                    in_=kv[:].rearrange("p (j kd) -> p j kd", j=S)[:, :, 0:P])
                vs = wp.tile([P, S * P], bf16, tag="vs")
                nc.scalar.copy(
                    out=vs[:].rearrange("p (j d) -> p j d", j=S),
                    in_=kv[:].rearrange("p (j kd) -> p j kd", j=S)[:, :, P:2 * P])

                # ---- Q = memT.T @ wqT + b_q ----
                qp = psmall.tile([P, 128], f32, tag="qp")
                q1 = nc.tensor.matmul(out=qp[:], lhsT=memT_s[:, t * P:(t + 1) * P],
                                      rhs=cb[:, wqT_o:wqT_o + 128],
                                      start=True, stop=False)
                q2 = nc.tensor.matmul(out=qp[:], lhsT=ones1b[:], rhs=bq_s[:],
                                      start=False, stop=True)
                add_dep_helper(q2.ins, q1.ins, sync=False, reason="psum group order")
                qs = sp.tile([P, P], bf16, tag="qs")
                nc.scalar.copy(out=qs[:], in_=qp[:])

                # ---- scores l[n,j,h] = sum_d Q[n,h,d] K[n,j,h,d] ----
                prod = wp.tile([P, S * P], bf16, tag="prod")
                nc.vector.tensor_tensor(
                    out=prod[:].rearrange("p (j d) -> p j d", j=S),
                    in0=ks[:].rearrange("p (j d) -> p j d", j=S),
                    in1=bcast(qs[:], S, 1), op=Alu.mult)
                lt = sp.tile([P, S * 2], f32, tag="lt")
                nc.vector.reduce_sum(
                    out=lt[:].rearrange("p (j h) -> p j h", j=S),
                    in_=prod[:].rearrange("p (j h d) -> p j h d", j=S, h=2),
                    axis=Ax.X)

                # ---- masked softmax over slots ----
                e = sp.tile([P, S * 2], bf16, tag="e")
                nc.scalar.activation(out=e[:], in_=lt[:], func=Act.Exp, scale=0.125)
                em = sp.tile([P, S * 2], bf16, tag="em")
                mslice = cb[:, mask_o + t * S: mask_o + (t + 1) * S]
                nc.vector.tensor_tensor(
                    out=em[:].rearrange("p (j h) -> p j h", j=S),
                    in0=e[:].rearrange("p (j h) -> p j h", j=S),
                    in1=bcast(mslice, 2, 2), op=Alu.mult)
                ssum = sp.tile([P, 2], f32, tag="ssum")
                nc.vector.reduce_sum(
                    out=ssum[:],
                    in_=em[:].rearrange("p (j h) -> p h j", j=S), axis=Ax.X)
                sc = sp.tile([P, 2], f32, tag="sc")
                nc.vector.tensor_scalar_max(sc[:], ssum[:], 1e-6)
                rr = sp.tile([P, 2], f32, tag="rr")
                nc.vector.reciprocal(rr[:], sc[:])

                # ---- outv_raw[n,h,d] = sum_j em[n,j,h] V[n,j,h,d] ----
                prod2 = wp.tile([P, S * P], f32, tag="prod2")
                nc.vector.tensor_tensor(
                    out=prod2[:].rearrange("p (j h d) -> p j h d", j=S, h=2),
                    in0=vs[:].rearrange("p (j h d) -> p j h d", j=S, h=2),
                    in1=bcast(em[:].rearrange("p (j h) -> p j h", j=S), DTIME, 3),
                    op=Alu.mult)
                outv = sp.tile([P, P], f32, tag="outv")
                nc.vector.reduce_sum(
                    out=outv[:],
                    in_=prod2[:].rearrange("p (j hd) -> p hd j", j=S),
                    axis=Ax.X)

                # ---- x = outv*rr + prev_mem (per head), then LN stats ----
                x = sp.tile([P, P], f32, tag="x")
                for h in range(2):
                    nc.vector.scalar_tensor_tensor(
                        out=x[:, h * 64:(h + 1) * 64],
                        in0=outv[:, h * 64:(h + 1) * 64],
                        scalar=rr[:, h:h + 1],
                        in1=cf[:, t * P + h * 64: t * P + (h + 1) * 64],
                        op0=Alu.mult, op1=Alu.add)
                bns = sp.tile([P, 6], f32, tag="bns")
                nc.vector.bn_stats(bns[:], x[:])
                bna = sp.tile([P, 2], f32, tag="bna")
                nc.vector.bn_aggr(bna[:], bns[:])
                negmu = sp.tile([P, 1], f32, tag="negmu")
                nc.vector.tensor_scalar_mul(negmu[:], bna[:, 0:1], -1.0)
                vpe = sp.tile([P, 1], f32, tag="vpe")
                nc.vector.tensor_scalar_add(vpe[:], bna[:, 1:2], 1e-5)

                # rstd = rsqrt(vpe): quake seed on gpsimd + 1 Newton iter
                # bits>>1 done as uint*0.5 (DVE computes ints in fp32; the
                # ±64-ulp rounding is irrelevant, Newton fixes the seed)
                y0 = sp.tile([P, 1], f32, tag="y0")
                tq = sp.tile([P, 1], u32, tag="tq")
                nc.vector.tensor_scalar_mul(tq[:], vpe[:].bitcast(u32), 0.5)
                nc.vector.tensor_tensor(
                    out=y0[:].bitcast(u32), in0=magicc[:], in1=tq[:],
                    op=Alu.subtract)
                t1 = sp.tile([P, 1], f32, tag="t1")
                nc.vector.tensor_tensor(out=t1[:], in0=y0[:], in1=y0[:], op=Alu.mult)
                t2 = sp.tile([P, 1], f32, tag="t2")
                nc.vector.tensor_tensor(out=t2[:], in0=t1[:], in1=vpe[:], op=Alu.mult)
                w1 = sp.tile([P, 1], f32, tag="w1")
                nc.vector.tensor_scalar(out=w1[:], in0=t2[:], scalar1=-0.5,
                                        scalar2=1.5, op0=Alu.mult, op1=Alu.add)
                rstd = sp.tile([P, 1], f32, tag="rstd")
                nc.vector.tensor_tensor(out=rstd[:], in0=y0[:], in1=w1[:], op=Alu.mult)

                xcb = sp.tile([P, P], bf16, tag="xcb")
                nc.vector.tensor_scalar_add(xcb[:], x[:], negmu[:])

                # ---- MLP: y = relu(rstd*(xc @ mlpT) + mlpb) ----
                xtp = pstg.tile([P, P], bf16, tag="stg")
                nc.tensor.transpose(out=xtp[:], in_=xcb[:], identity=identb[:])
                xts = sp.tile([P, P], bf16, tag="xts")
                nc.scalar.copy(out=xts[:], in_=xtp[:])
                yp = psmall.tile([P, 128], f32, tag="qp")
                nc.tensor.matmul(out=yp[:], lhsT=xts[:],
                                 rhs=cb[:, mlpT_o:mlpT_o + 128],
                                 start=True, stop=True)
                yb = sp.tile([P, P], f32, tag="yb")
                nc.vector.scalar_tensor_tensor(
                    out=yb[:], in0=yp[:], scalar=rstd[:],
                    in1=cf[:, NT * P:NT * P + 128],
                    op0=Alu.mult, op1=Alu.add)
                upd = sp.tile([P, P], f32, tag="upd")
                nc.vector.tensor_scalar_max(upd[:], yb[:], 0.0)
                nc.sync.dma_start(out=out[t * P:(t + 1) * P, :], in_=upd[:])

    nc.compile()
    return nc


def _host_prep(inputs):
    """Build per-core in_maps from full inputs."""
    import ml_dtypes
    bf16 = ml_dtypes.bfloat16

    memory = np.asarray(inputs["memory"], dtype=np.float32)
    mail = np.asarray(inputs["mail"], dtype=np.float32)
    mail_ts = np.asarray(inputs["mail_ts"], dtype=np.float32)
    mail_ptr = np.asarray(inputs["mail_ptr"], dtype=np.int32)
    mail_count = np.asarray(inputs["mail_count"], dtype=np.int32)
    nodes = np.asarray(inputs["nodes"], dtype=np.int32)
    curr_ts = np.asarray(inputs["curr_ts"], dtype=np.float32)
    w_k = np.asarray(inputs["w_k"], dtype=np.float32)
    w_v = np.asarray(inputs["w_v"], dtype=np.float32)
    b_v = np.asarray(inputs["b_v"], dtype=np.float32)
    w_q = np.asarray(inputs["w_q"], dtype=np.float32)
    b_q = np.asarray(inputs["b_q"], dtype=np.float32)
    mlp_w = np.asarray(inputs["mlp_w"], dtype=np.float32)
    mlp_b = np.asarray(inputs["mlp_b"], dtype=np.float32)
    ln_g = np.asarray(inputs["ln_g"], dtype=np.float32)
    ln_b = np.asarray(inputs["ln_b"], dtype=np.float32)
    time_w = np.asarray(inputs["time_w"], dtype=np.float32)
    time_b = np.asarray(inputs["time_b"], dtype=np.float32)

    key = ("mail2d", id(inputs.get("mail")))
    mail2d = np.ascontiguousarray(
        mail[:, :, 0:DMD].reshape(NN, S * DMD)).astype(bf16)

    # shared weights
    wkv = np.concatenate([w_k.T, w_v.T], axis=1).astype(np.float32)  # [352,256]
    bias_row = np.concatenate([np.zeros(P, np.float32), b_v])[None, :]
    wkv2 = np.concatenate(
        [wkv[256:352], bias_row, np.zeros((31, 256), np.float32)], axis=0)
    wqT = w_q.T
    mlpT = ln_g[:, None] * mlp_w.T
    mlpb = (ln_b @ mlp_w.T + mlp_b)

    in_maps = []
    jj = np.arange(S, dtype=np.int32)
    for c in range(NCORES):
        sl = slice(c * B_CORE, (c + 1) * B_CORE)
        nd = nodes[sl]
        ct = curr_ts[sl]
        mts = mail_ts[nd]                        # [3072, 10]
        dtv = ct[:, None] - mts                  # [3072, 10]
        tf = np.cos(dtv[:, :, None] * time_w + time_b).astype(np.float32)
        # feature-major: [NT, 64, 10, 128]
        tfT = tf.reshape(NT, P, S, DTIME).transpose(0, 3, 2, 1)
        # last 32 mail dims, feature-major: [NT, 32, 10, 128]
        mc2 = mail[nd][:, :, DMD:DM].reshape(NT, P, S, DM - DMD) \
            .transpose(0, 3, 2, 1)
        tfd = np.concatenate(
            [mc2, tfT, np.ones((NT, 1, S, P), np.float32)], axis=1
        ).reshape(NT, 97, S * P).astype(bf16)

        ptr = mail_ptr[nd]
        cnt = mail_count[nd]
        mask = (((ptr[:, None] - 1 - jj) % S) < cnt[:, None]).astype(np.float32)
        maskf = mask.reshape(NT, P, S).transpose(1, 0, 2).reshape(P, NT * S)

        pm = memory[nd]                          # [3072, 128]
        prevm = pm.reshape(NT, P, D).transpose(1, 0, 2).reshape(P, NT * D)
        memT = pm.reshape(NT, P, D).transpose(2, 0, 1).reshape(D, NT * P)
        idxs = np.ascontiguousarray(nd.reshape(NT, P).T)

        constb = np.concatenate(
            [memT, wkv[0:128], wkv[128:256], wkv2, wqT, mlpT, maskf],
            axis=1).astype(bf16)
        constf = np.concatenate(
            [prevm, np.tile(mlpb[None, :], (P, 1))], axis=1).astype(np.float32)

        in_maps.append({
            "mail": mail2d,
            "idxs": idxs,
            "tfd": np.ascontiguousarray(tfd),
            "constb": np.ascontiguousarray(constb),
            "constf": np.ascontiguousarray(constf),
            "bq": b_q[None, :].astype(bf16),
        })
    return in_maps


def kernel(**inputs):
    from concourse.bass_utils import run_bass_kernel_spmd

    nodes = np.asarray(inputs["nodes"], dtype=np.int32)
    curr_ts = np.asarray(inputs["curr_ts"], dtype=np.float32)
    memory = np.asarray(inputs["memory"], dtype=np.float32)
    memory_ts = np.asarray(inputs["memory_ts"], dtype=np.float32)
    assert nodes.shape[0] == N_TOTAL

    if "nc" not in _CACHE:
        _CACHE["nc"] = _build()
    nc = _CACHE["nc"]

    in_maps = _host_prep(inputs)
    import os
    trace = bool(int(os.environ.get("KERNEL_TRACE", "0")))
    res = run_bass_kernel_spmd(nc, in_maps, core_ids=list(range(NCORES)),
                               trace=trace)
    _CACHE["last_result"] = res

    upd = np.concatenate([r["out"] for r in res.results], axis=0)  # [24576,128]

    bb = N_TOTAL // 3
    pos = nodes[:2 * bb]
    new_memory = memory.copy()
    new_memory[pos] = upd[:2 * bb]
    new_memory_ts = memory_ts.copy()
    new_memory_ts[pos] = curr_ts[:2 * bb]
    return upd, new_memory, new_memory_ts


# revision 19
# speedup vs baseline: 2.7108x; 1.5181x over previous
"""Trainium2 Bass kernel for nn_AtlasApan (TGN scatter-memory update).

Strategy (8 NeuronCores, SPMD):
  - Batch dim (24576) sharded contiguously: 3072 nodes/core = 24 tiles of 128.
  - `mail` (first 256 msg dims, bf16) replicated in each core's HBM; the
    per-node mailbox gather runs on-device via indirect DMA.
  - Host precomputes small per-node tensors: time features + last 32 mail
    dims, pre-transposed (feature-major), slot-validity masks (the
    reference's modulo reorder reduces to a mask because softmax attention
    is permutation invariant), prev-memory in both layouts.
  - Math folds: b_k cancels in softmax; LayerNorm affine folds into the MLP
    weights; rstd commutes through the MLP matmul (applied in a fused
    scalar_tensor_tensor after it); K/V share one fused matmul
    (rhs = [w_k.T | w_v.T]); rsqrt via bit-trick + Newton (avoids ACT
    table-set thrash between Exp and Ln).
Per 128-node tile on device (all matmul operands bf16, PSUM fp32):
  gather mail -> PE-transpose to feature-major msgT -> K|V matmuls into
  PSUM (one accumulation group per 2KB bank) -> copy K,V to SBUF bf16 ->
  per-node scores via DVE mul+segmented reduce -> masked softmax (exp on
  ACT, no max-sub needed; empty rows renormalize to 0) -> attn-weighted V
  -> +prev_mem -> LN stats via bn_stats -> MLP + relu.
"""

import numpy as np

P = 128
NT = 24            # tiles per core
S = 10             # mail slots
DM = 288           # mail message dim
DMD = 256          # device-gathered mail dims (rest rides with tfd)
DTIME = 64         # time-encoding dim
D = 128            # embed dim
NN = 50000         # nodes in memory
NCORES = 8
B_CORE = NT * P    # 3072
N_TOTAL = NCORES * B_CORE  # 24576

_CACHE = {}


def _build():
    import concourse.mybir as mybir
    from concourse.bacc import Bacc
    from concourse.tile import TileContext
    from concourse.tile_rust import add_dep_helper
    from concourse.masks import make_identity
    from concourse.bass import IndirectOffsetOnAxis, AP

    dt = mybir.dt
    f32 = dt.float32
    bf16 = dt.bfloat16
    i32 = dt.int32
    u32 = dt.uint32
    Alu = mybir.AluOpType
    Act = mybir.ActivationFunctionType
    Ax = mybir.AxisListType

    nc = Bacc()

    mail = nc.dram_tensor("mail", [NN, S * DMD], bf16, kind="ExternalInput")
    idxs = nc.dram_tensor("idxs", [P, NT], i32, kind="ExternalInput")
    # [tile, 32 mail-c2 rows + 64 time rows + 1 ones row, 10*128]
    tfd = nc.dram_tensor("tfd", [NT, 97, S * P], bf16, kind="ExternalInput")
    # packed bf16 consts: memT | wkv0 | wkv1 | wkv2(pad128) | wqT | mlpT | mask
    CB = NT * P + 256 + 256 + 256 + 128 + 128 + NT * S  # 4336
    constb = nc.dram_tensor("constb", [P, CB], bf16, kind="ExternalInput")
    # packed f32 consts: prevm | mlpbrep
    CF = NT * P + 128
    constf = nc.dram_tensor("constf", [P, CF], f32, kind="ExternalInput")
    bq = nc.dram_tensor("bq", [1, P], bf16, kind="ExternalInput")
    out = nc.dram_tensor("out", [NT * P, P], f32, kind="ExternalOutput")

    def bcast(ap, n, pos):
        """Insert a broadcast (step 0, count n) free dim at position pos."""
        dims = [list(d) for d in ap.ap]
        dims.insert(pos, [0, n])
        return AP(ap.tensor, ap.offset, dims)

    MAGIC = 0x5F3759DF

    with TileContext(nc) as tc:
        with tc.tile_pool(name="const", bufs=1) as cp, \
             tc.tile_pool(name="gat", bufs=3) as gp, \
             tc.tile_pool(name="work", bufs=2) as wp, \
             tc.tile_pool(name="small", bufs=3) as sp, \
             tc.tile_pool(name="stgp", bufs=2, space="PSUM") as pstg, \
             tc.tile_pool(name="smp", bufs=1, space="PSUM") as psmall, \
             tc.tile_pool(name="kvp", bufs=1, space="PSUM") as pkv:

            identb = cp.tile([P, P], bf16)
            make_identity(nc, identb[:])
            ones1b = cp.tile([1, P], bf16)
            nc.gpsimd.memset(ones1b[:], 1.0)
            zeroc = cp.tile([P, 1], f32)
            nc.gpsimd.memset(zeroc[:], 0.0)
            nc.const_aps.aps[(f32, 0.0)] = zeroc[:]
            magicc = cp.tile([P, 1], u32)
            nc.gpsimd.memset(magicc[:], MAGIC)

            cb = cp.tile([P, CB], bf16)
            nc.sync.dma_start(out=cb[:], in_=constb[:])
            cf = cp.tile([P, CF], f32)
            nc.sync.dma_start(out=cf[:], in_=constf[:])
            bq_s = cp.tile([1, P], bf16)
            nc.sync.dma_start(out=bq_s[:], in_=bq[:])
            idx_s = cp.tile([P, NT], i32)
            nc.sync.dma_start(out=idx_s[:], in_=idxs[:])

            o = NT * P
            memT_s = cb
            wkv0_o = o; o += 256
            wkv1_o = o; o += 256
            wkv2_o = o; o += 256
            wqT_o = o; o += 128
            mlpT_o = o; o += 128
            mask_o = o
            wkv_os = (wkv0_o, wkv1_o, wkv2_o)

            def emit_gather(t):
                mailg = gp.tile([P, S * DMD], bf16, tag="mailg", name=f"mailg{t}")
                nc.gpsimd.indirect_dma_start(
                    out=mailg[:],
                    out_offset=None,
                    in_=mail[:],
                    in_offset=IndirectOffsetOnAxis(ap=idx_s[:, t:t + 1], axis=0),
                )
                return mailg

            def alloc_msgT(t):
                msgT0 = wp.tile([P, S * P], bf16, tag="m0", name=f"m0_{t}")
                msgT1 = wp.tile([P, S * P], bf16, tag="m1", name=f"m1_{t}")
                msgT2 = gp.tile([97, S * P], bf16, tag="m2", name=f"m2_{t}")
                nc.sync.dma_start(out=msgT2[:], in_=tfd[t])
                return msgT0, msgT1, msgT2

            # transpose-group emitters: each fills a PSUM bank then copies
            # to SBUF (one accumulation group per bank, explicit ordering)
            TPLANS = [
                ([(0, list(range(0, 8)), 0)],
                 [(0, 0, 0, 1024)]),
                ([(0, [8, 9], 0), (1, list(range(0, 6)), 256)],
                 [(0, 1024, 0, 256), (1, 0, 256, 768)]),
                ([(1, list(range(6, 10)), 0)],
                 [(1, 768, 0, 512)]),
            ]

            def transpose_emitters(mailg, msgTs):
                def mk(grp, cps):
                    def em():
                        stg = pstg.tile([P, 1024], bf16, tag="stg", name="stg")
                        nmm = sum(len(sl) for _, sl, _ in grp)
                        k = 0
                        prev = None
                        for c, sl, base in grp:
                            for ji, j in enumerate(sl):
                                mi = nc.tensor.matmul(
                                    out=stg[:, base + ji * P: base + (ji + 1) * P],
                                    lhsT=mailg[:, j * DMD + c * 128:
                                               j * DMD + c * 128 + 128],
                                    rhs=identb[:],
                                    is_transpose=True,
                                    start=(k == 0),
                                    stop=(k == nmm - 1),
                                )
                                if prev is not None:
                                    add_dep_helper(mi.ins, prev.ins, sync=False,
                                                   reason="psum bank order")
                                prev = mi
                                k += 1
                        for di, dco, sco, wdt in cps:
                            nc.scalar.copy(out=msgTs[di][:, dco:dco + wdt],
                                           in_=stg[:, sco:sco + wdt])
                    return em
                return [mk(g, c) for g, c in TPLANS]

            def kv_bank_emitters(kv, msgTs):
                srcs = [(msgTs[0][:], P), (msgTs[1][:], P), (msgTs[2][:], 97)]
                def mk(b):
                    def em():
                        prev = None
                        for j in (2 * b, 2 * b + 1):
                            for c, (mt, rows) in enumerate(srcs):
                                mi = nc.tensor.matmul(
                                    out=kv[:, j * 256:(j + 1) * 256],
                                    lhsT=mt[0:rows, j * P:(j + 1) * P],
                                    rhs=cb[0:rows, wkv_os[c]:wkv_os[c] + 256],
                                    start=(j == 2 * b and c == 0),
                                    stop=(j == 2 * b + 1 and c == 2),
                                )
                                if prev is not None:
                                    add_dep_helper(mi.ins, prev.ins, sync=False,
                                                   reason="psum bank order")
                                prev = mi
                    return em
                return [mk(b) for b in range(S // 2)]

            # software pipeline: transposes of tile t+1 interleave with the
            # K/V matmuls of tile t so PE sees a dense real-matmul stream
            mail_t = {0: emit_gather(0)}
            if NT > 1:
                mail_t[1] = emit_gather(1)
            msgT_t = {0: alloc_msgT(0)}
            for em in transpose_emitters(mail_t[0], msgT_t[0]):
                em()

            for t in range(NT):
                if t + 2 < NT:
                    mail_t[t + 2] = emit_gather(t + 2)
                tems = []
                if t + 1 < NT:
                    msgT_t[t + 1] = alloc_msgT(t + 1)
                    tems = transpose_emitters(mail_t[t + 1], msgT_t[t + 1])

                msgT0, msgT1, msgT2 = msgT_t[t]
                kv = pkv.tile([P, S * 256], f32, tag="kv")
                kvems = kv_bank_emitters(kv, msgT_t[t])
                for i in range(5):
                    kvems[i]()
                    if i < len(tems):
                        tems[i]()

                # copy K, V out to SBUF as bf16 (frees PSUM, enables 2x DVE)
                ks = wp.tile([P, S * P], bf16, tag="ks")
                nc.scalar.copy(
                    out=ks[:].rearrange("p (j d) -> p j d", j=S),
                    in_=kv[:].rearrange("p (j kd) -> p j kd", j=S)[:, :, 0:P])
                vs = wp.tile([P, S * P], bf16, tag="vs")
                nc.scalar.copy(
                    out=vs[:].rearrange("p (j d) -> p j d", j=S),
                    in_=kv[:].rearrange("p (j kd) -> p j kd", j=S)[:, :, P:2 * P])

                # ---- Q = memT.T @ wqT + b_q ----
                qp = psmall.tile([P, 128], f32, tag="qp")
                q1 = nc.tensor.matmul(out=qp[:], lhsT=memT_s[:, t * P:(t + 1) * P],
                                      rhs=cb[:, wqT_o:wqT_o + 128],
                                      start=True, stop=False)
                q2 = nc.tensor.matmul(out=qp[:], lhsT=ones1b[:], rhs=bq_s[:],
                                      start=False, stop=True)
                add_dep_helper(q2.ins, q1.ins, sync=False, reason="psum group order")
                qs = sp.tile([P, P], bf16, tag="qs")
                nc.scalar.copy(out=qs[:], in_=qp[:])

                # ---- scores l[n,j,h] = sum_d Q[n,h,d] K[n,j,h,d] ----
                prod = wp.tile([P, S * P], bf16, tag="prod")
                nc.vector.tensor_tensor(
                    out=prod[:].rearrange("p (j d) -> p j d", j=S),
                    in0=ks[:].rearrange("p (j d) -> p j d", j=S),
                    in1=bcast(qs[:], S, 1), op=Alu.mult)
                lt = sp.tile([P, S * 2], f32, tag="lt")
                nc.vector.reduce_sum(
                    out=lt[:].rearrange("p (j h) -> p j h", j=S),
                    in_=prod[:].rearrange("p (j h d) -> p j h d", j=S, h=2),
                    axis=Ax.X)

                # ---- masked softmax over slots ----
                e = sp.tile([P, S * 2], bf16, tag="e")
                nc.scalar.activation(out=e[:], in_=lt[:], func=Act.Exp, scale=0.125)
                em = sp.tile([P, S * 2], bf16, tag="em")
                mslice = cb[:, mask_o + t * S: mask_o + (t + 1) * S]
                nc.vector.tensor_tensor(
                    out=em[:].rearrange("p (j h) -> p j h", j=S),
                    in0=e[:].rearrange("p (j h) -> p j h", j=S),
                    in1=bcast(mslice, 2, 2), op=Alu.mult)
                ssum = sp.tile([P, 2], f32, tag="ssum")
                nc.vector.reduce_sum(
                    out=ssum[:],
                    in_=em[:].rearrange("p (j h) -> p h j", j=S), axis=Ax.X)
                sc = sp.tile([P, 2], f32, tag="sc")
                nc.vector.tensor_scalar_max(sc[:], ssum[:], 1e-6)
                rr = sp.tile([P, 2], f32, tag="rr")
                nc.vector.reciprocal(rr[:], sc[:])

                # ---- outv_raw[n,h,d] = sum_j em[n,j,h] V[n,j,h,d] ----
                prod2 = wp.tile([P, S * P], f32, tag="prod2")
                nc.vector.tensor_tensor(
                    out=prod2[:].rearrange("p (j h d) -> p j h d", j=S, h=2),
                    in0=vs[:].rearrange("p (j h d) -> p j h d", j=S, h=2),
                    in1=bcast(em[:].rearrange("p (j h) -> p j h", j=S), DTIME, 3),
                    op=Alu.mult)
                outv = sp.tile([P, P], f32, tag="outv")
                nc.vector.reduce_sum(
                    out=outv[:],
                    in_=prod2[:].rearrange("p (j hd) -> p hd j", j=S),
                    axis=Ax.X)

                # ---- x = outv*rr + prev_mem (per head), then LN stats ----
                x = sp.tile([P, P], f32, tag="x")
                for h in range(2):
                    nc.vector.scalar_tensor_tensor(
                        out=x[:, h * 64:(h + 1) * 64],
                        in0=outv[:, h * 64:(h + 1) * 64],
                        scalar=rr[:, h:h + 1],
                        in1=cf[:, t * P + h * 64: t * P + (h + 1) * 64],
                        op0=Alu.mult, op1=Alu.add)
                bns = sp.tile([P, 6], f32, tag="bns")
                nc.vector.bn_stats(bns[:], x[:])
                bna = sp.tile([P, 2], f32, tag="bna")
                nc.vector.bn_aggr(bna[:], bns[:])
                negmu = sp.tile([P, 1], f32, tag="negmu")
                nc.vector.tensor_scalar_mul(negmu[:], bna[:, 0:1], -1.0)
                vpe = sp.tile([P, 1], f32, tag="vpe")
                nc.vector.tensor_scalar_add(vpe[:], bna[:, 1:2], 1e-5)

                # rstd = rsqrt(vpe): quake seed on gpsimd + 1 Newton iter
                # bits>>1 done as uint*0.5 (DVE computes ints in fp32; the
                # ±64-ulp rounding is irrelevant, Newton fixes the seed)
                y0 = sp.tile([P, 1], f32, tag="y0")
                tq = sp.tile([P, 1], u32, tag="tq")
                nc.vector.tensor_scalar_mul(tq[:], vpe[:].bitcast(u32), 0.5)
                nc.vector.tensor_tensor(
                    out=y0[:].bitcast(u32), in0=magicc[:], in1=tq[:],
                    op=Alu.subtract)
                t1 = sp.tile([P, 1], f32, tag="t1")
                nc.vector.tensor_tensor(out=t1[:], in0=y0[:], in1=y0[:], op=Alu.mult)
                t2 = sp.tile([P, 1], f32, tag="t2")
                nc.vector.tensor_tensor(out=t2[:], in0=t1[:], in1=vpe[:], op=Alu.mult)
                w1 = sp.tile([P, 1], f32, tag="w1")
                nc.vector.tensor_scalar(out=w1[:], in0=t2[:], scalar1=-0.5,
                                        scalar2=1.5, op0=Alu.mult, op1=Alu.add)
                rstd = sp.tile([P, 1], f32, tag="rstd")
                nc.vector.tensor_tensor(out=rstd[:], in0=y0[:], in1=w1[:], op=Alu.mult)

                xcb = sp.tile([P, P], bf16, tag="xcb")
                nc.vector.tensor_scalar_add(xcb[:], x[:], negmu[:])

                # ---- MLP: y = relu(rstd*(xc @ mlpT) + mlpb) ----
                xtp = pstg.tile([P, P], bf16, tag="stg")
                nc.tensor.transpose(out=xtp[:], in_=xcb[:], identity=identb[:])
                xts = sp.tile([P, P], bf16, tag="xts")
                nc.scalar.copy(out=xts[:], in_=xtp[:])
                yp = psmall.tile([P, 128], f32, tag="qp")
                nc.tensor.matmul(out=yp[:], lhsT=xts[:],
                                 rhs=cb[:, mlpT_o:mlpT_o + 128],
                                 start=True, stop=True)
                yb = sp.tile([P, P], f32, tag="yb")
                nc.vector.scalar_tensor_tensor(
                    out=yb[:], in0=yp[:], scalar=rstd[:],
                    in1=cf[:, NT * P:NT * P + 128],
                    op0=Alu.mult, op1=Alu.add)
                upd = sp.tile([P, P], f32, tag="upd")
                nc.vector.tensor_scalar_max(upd[:], yb[:], 0.0)
                nc.sync.dma_start(out=out[t * P:(t + 1) * P, :], in_=upd[:])

    nc.compile()
    return nc


def _host_prep(inputs):
    """Build per-core in_maps from full inputs."""
    import ml_dtypes
    bf16 = ml_dtypes.bfloat16

    memory = np.asarray(inputs["memory"], dtype=np.float32)
    mail = np.asarray(inputs["mail"], dtype=np.float32)
    mail_ts = np.asarray(inputs["mail_ts"], dtype=np.float32)
    mail_ptr = np.asarray(inputs["mail_ptr"], dtype=np.int32)
    mail_count = np.asarray(inputs["mail_count"], dtype=np.int32)
    nodes = np.asarray(inputs["nodes"], dtype=np.int32)
    curr_ts = np.asarray(inputs["curr_ts"], dtype=np.float32)
    w_k = np.asarray(inputs["w_k"], dtype=np.float32)
    w_v = np.asarray(inputs["w_v"], dtype=np.float32)
    b_v = np.asarray(inputs["b_v"], dtype=np.float32)
    w_q = np.asarray(inputs["w_q"], dtype=np.float32)
    b_q = np.asarray(inputs["b_q"], dtype=np.float32)
    mlp_w = np.asarray(inputs["mlp_w"], dtype=np.float32)
    mlp_b = np.asarray(inputs["mlp_b"], dtype=np.float32)
    ln_g = np.asarray(inputs["ln_g"], dtype=np.float32)
    ln_b = np.asarray(inputs["ln_b"], dtype=np.float32)
    time_w = np.asarray(inputs["time_w"], dtype=np.float32)
    time_b = np.asarray(inputs["time_b"], dtype=np.float32)

    key = ("mail2d", id(inputs.get("mail")))
    mail2d = np.ascontiguousarray(
        mail[:, :, 0:DMD].reshape(NN, S * DMD)).astype(bf16)

    # shared weights
    wkv = np.concatenate([w_k.T, w_v.T], axis=1).astype(np.float32)  # [352,256]
    bias_row = np.concatenate([np.zeros(P, np.float32), b_v])[None, :]
    wkv2 = np.concatenate(
        [wkv[256:352], bias_row, np.zeros((31, 256), np.float32)], axis=0)
    wqT = w_q.T
    mlpT = ln_g[:, None] * mlp_w.T
    mlpb = (ln_b @ mlp_w.T + mlp_b)

    in_maps = []
    jj = np.arange(S, dtype=np.int32)
    for c in range(NCORES):
        sl = slice(c * B_CORE, (c + 1) * B_CORE)
        nd = nodes[sl]
        ct = curr_ts[sl]
        mts = mail_ts[nd]                        # [3072, 10]
        dtv = ct[:, None] - mts                  # [3072, 10]
        tf = np.cos(dtv[:, :, None] * time_w + time_b).astype(np.float32)
        # feature-major: [NT, 64, 10, 128]
        tfT = tf.reshape(NT, P, S, DTIME).transpose(0, 3, 2, 1)
        # last 32 mail dims, feature-major: [NT, 32, 10, 128]
        mc2 = mail[nd][:, :, DMD:DM].reshape(NT, P, S, DM - DMD) \
            .transpose(0, 3, 2, 1)
        tfd = np.concatenate(
            [mc2, tfT, np.ones((NT, 1, S, P), np.float32)], axis=1
        ).reshape(NT, 97, S * P).astype(bf16)

        ptr = mail_ptr[nd]
        cnt = mail_count[nd]
        mask = (((ptr[:, None] - 1 - jj) % S) < cnt[:, None]).astype(np.float32)
        maskf = mask.reshape(NT, P, S).transpose(1, 0, 2).reshape(P, NT * S)

        pm = memory[nd]                          # [3072, 128]
        prevm = pm.reshape(NT, P, D).transpose(1, 0, 2).reshape(P, NT * D)
        memT = pm.reshape(NT, P, D).transpose(2, 0, 1).reshape(D, NT * P)
        idxs = np.ascontiguousarray(nd.reshape(NT, P).T)

        constb = np.concatenate(
            [memT, wkv[0:128], wkv[128:256], wkv2, wqT, mlpT, maskf],
            axis=1).astype(bf16)
        constf = np.concatenate(
            [prevm, np.tile(mlpb[None, :], (P, 1))], axis=1).astype(np.float32)

        in_maps.append({
            "mail": mail2d,
            "idxs": idxs,
            "tfd": np.ascontiguousarray(tfd),
            "constb": np.ascontiguousarray(constb),
            "constf": np.ascontiguousarray(constf),
            "bq": b_q[None, :].astype(bf16),
        })
    return in_maps


def kernel(**inputs):
    from concourse.bass_utils import run_bass_kernel_spmd

    nodes = np.asarray(inputs["nodes"], dtype=np.int32)
    curr_ts = np.asarray(inputs["curr_ts"], dtype=np.float32)
    memory = np.asarray(inputs["memory"], dtype=np.float32)
    memory_ts = np.asarray(inputs["memory_ts"], dtype=np.float32)
    assert nodes.shape[0] == N_TOTAL

    if "nc" not in _CACHE:
        _CACHE["nc"] = _build()
    nc = _CACHE["nc"]

    in_maps = _host_prep(inputs)
    import os
    trace = bool(int(os.environ.get("KERNEL_TRACE", "0")))
    res = run_bass_kernel_spmd(nc, in_maps, core_ids=list(range(NCORES)),
                               trace=trace)
    _CACHE["last_result"] = res

    upd = np.concatenate([r["out"] for r in res.results], axis=0)  # [24576,128]

    bb = N_TOTAL // 3
    pos = nodes[:2 * bb]
    new_memory = memory.copy()
    new_memory[pos] = upd[:2 * bb]
    new_memory_ts = memory_ts.copy()
    new_memory_ts[pos] = curr_ts[:2 * bb]
    return upd, new_memory, new_memory_ts
